# revision 8
# baseline (speedup 1.0000x reference)
"""MoE (top-2 of 8 experts, SwiGLU) on 8 Trainium2 NeuronCores.

Strategy (expert-parallel, per the sharding hint):
  - Host computes the router (tiny: [2048,1024]@[1024,8]) and the top-2
    dispatch: for each expert e, the list of tokens routed to it and their
    combine weights. This IS the sharding step — each core's input shard is
    "its expert's weights + its expert's tokens".
  - Core e runs the expert MLP for its ~512 tokens:
        hT = w1[e] @ x_eT            (gate/up fused, [4096, C])
        yT = silu(hT_gate) * hT_up   ([2048, C])
        oT = (w2[e] @ yT) * combine  ([1024, C])
    GEMM1 in bf16, GEMM2 in float32r (same PE speed at moving-dim >=256),
    fp32 PSUM accumulation throughout; activations fp32.
  - Host scatter-adds the per-expert outputs back to token order (unshard).

Layouts keep tokens on the PSUM free dim everywhere so no on-device
transposes are needed; weights are pre-transposed on the host.
"""

import sys

sys.path.insert(0, "/opt/trn_rl_repo")

import numpy as np
import ml_dtypes

import concourse.bass as bass  # noqa: F401  (bass must import before tile)
import concourse.tile as tile
from concourse import bacc, mybir
from concourse.bass_utils import run_bass_kernel_spmd

T = 2048
H = 1024
INTER = 2048
E = 8
TOPK = 2
N_CORES = 8
P = 128

DT = mybir.dt.bfloat16
NP_DT = ml_dtypes.bfloat16

# GEMM2 dtype: bf16 (error ~4e-3 vs f32r's ~3e-3 — both far under the 2e-2
# gate) halves the w2 DMA stream (4.2MB vs 8.4MB) and avoids f32r's 4-byte
# weight loads, freeing DMA headroom in the steady-state loop.
G2_F32R = False

_PROGRAM_CACHE = {}    # c_total -> compiled Bacc program (reused across calls)

KH = H // P            # 8  k-tiles for GEMM1 (contract over H)
KI = INTER // P        # 16 k-tiles for GEMM2 (contract over INTER)
NPAIR = INTER // P     # 16 gate/up pairs
NH = H // P            # 8  output h-tiles


def _route(x, router_w):
    """Replicates the reference router in fp32 numpy.

    Returns per-expert (token_indices, combine_weights)."""
    gating = (x @ router_w.T).astype(np.float32)              # [T, E]
    m = gating.max(axis=1, keepdims=True)
    p = np.exp(gating - m, dtype=np.float32)
    probs = p / p.sum(axis=1, keepdims=True)
    order = np.argsort(-probs, axis=1, kind="stable")         # ties -> lower idx
    sel = order[:, :TOPK]                                     # [T, K]
    topw = np.take_along_axis(probs, sel, axis=1)             # [T, K]

    idxs, wts = [], []
    for e in range(E):
        m_e = sel == e                                        # [T, K]
        rows = np.nonzero(m_e.any(axis=1))[0]
        idxs.append(rows.astype(np.int64))
        wts.append(topw[m_e].astype(np.float32))              # aligned with rows
    return idxs, wts


def _chunks(c):
    """Split c tokens into near-equal chunks of <=512 (PSUM bank limit).

    Chunks are kept >=256 where possible: below that, float32r matmuls drop
    to 1/4 rate and LDWEIGHTS (~107 ns) stops hiding under the matmul."""
    n = -(-c // 512)
    base = -(-(-(-c // n)) // 4) * 4                          # ceil(c/n) to mult of 4
    sizes = []
    left = c
    for _ in range(n - 1):
        sizes.append(base)
        left -= base
    sizes.append(left)
    return [s for s in sizes if s > 0]


def _build_program(c_total, loop_n=0):
    """One SPMD program: the expert MLP for c_total (padded) tokens.

    loop_n > 0 wraps the body in an on-device For_i loop running it loop_n
    times (used only by the perf harness to measure the per-iteration slope;
    the graded path uses loop_n=0 = straight-line body)."""
    nc = bacc.Bacc("TRN2", target_bir_lowering=False, debug=False,
                   num_devices=N_CORES)
    f32 = mybir.dt.float32
    xt_d = nc.dram_tensor("xt", [H, c_total], DT, kind="ExternalInput").ap()
    w1t_d = nc.dram_tensor("w1t", [H, 2 * INTER], DT, kind="ExternalInput").ap()
    dt2 = mybir.dt.float32r if G2_F32R else DT
    w2t_d = nc.dram_tensor("w2t", [INTER, H], dt2, kind="ExternalInput").ap()
    sc_d = nc.dram_tensor("scale", [P, c_total], f32, kind="ExternalInput").ap()
    out_d = nc.dram_tensor("out", [H, c_total], f32, kind="ExternalOutput").ap()

    chunk_sizes = _chunks(c_total)

    from contextlib import ExitStack
    with tile.TileContext(nc) as tc, ExitStack() as ctx:
        wpool = ctx.enter_context(tc.tile_pool(name="weights", bufs=1))
        xpool = ctx.enter_context(tc.tile_pool(name="xt", bufs=1))
        ypool = ctx.enter_context(tc.tile_pool(name="yt",
                                               bufs=1 if G2_F32R else 2))
        apool = ctx.enter_context(tc.tile_pool(name="act", bufs=2))
        opool = ctx.enter_context(tc.tile_pool(name="ot", bufs=2))
        pgpool = ctx.enter_context(tc.tile_pool(name="psg", bufs=3, space="PSUM"))
        pupool = ctx.enter_context(tc.tile_pool(name="psu", bufs=3, space="PSUM"))
        popool = ctx.enter_context(tc.tile_pool(name="pso", bufs=2, space="PSUM"))

        if loop_n:
            # staggered_reset: per-engine stage-preamble semaphore resets
            # instead of an all-engine barrier at the back edge — lets the
            # DMA queues run ahead into the next iteration's prologue while
            # the PE drains the current one.
            loop = ctx.enter_context(tc.For_i(
                0, loop_n, 1, staggered_reset=True,
                hint_engines=(mybir.EngineType.PE, mybir.EngineType.SP,
                              mybir.EngineType.Activation, mybir.EngineType.DVE)))

        # ---- PE warmup ----
        # ~4.5 us of dependency-free matmuls on an (uninitialized) scratch
        # tile: the PE HAM clock-gate warms during the initial DMA wait
        # instead of throttling the first real matmuls, and the loop's
        # per-iteration DMA prologue is masked by them. The product is
        # never read, so garbage input is fine.
        warm_sb = xpool.tile([P, P], DT, tag="warm")
        nc.vector.memset(warm_sb[:, 0:1], 0.0)
        ps_w = popool.tile([P, P], f32, tag="pso", name="ps_warm")
        for _ in range(44 if not loop_n else 4):
            nc.tensor.matmul(ps_w[:], lhsT=warm_sb[:], rhs=warm_sb[:],
                             start=True, stop=True)

        # ---- input loads ----
        # One merged DMA per logical tensor/piece: the HWDGE prep cost is
        # per-instruction (~625 ns, serialized), so many small DMAs stall the
        # PE at startup.
        NW1P = 8
        W1PC = 2 * INTER // NW1P  # 512

        # xt: 3 DMAs (chunk-1 columns first, split by k — they gate the
        # first matmuls)
        xt_t = xpool.tile([P, KH, c_total], DT, tag="xt")
        xt_view = xt_d.rearrange("(k p) c -> p k c", p=P)
        c1 = chunk_sizes[0]
        nc.sync.dma_start(out=xt_t[:, :KH // 2, :c1],
                          in_=xt_view[:, :KH // 2, :c1])
        xt_sb = [xt_t[:, k, :] for k in range(KH)]

        # first 256 cols of w1 for k=0..3 — unblocks the first two pairs
        w1_0a = wpool.tile([P, KH, 2 * P], DT, tag="w1_0a")
        w1_0a_view = w1t_d[:, :2 * P].rearrange("(k p) c -> p k c", p=P)
        nc.sync.dma_start(out=w1_0a[:, :KH // 2, :],
                          in_=w1_0a_view[:, :KH // 2, :])

        # w1t column pieces (each with all 8 k-tiles), in PE consumption
        # order (gate piece p feeds pairs 4p..4p+3, paired with up piece p+4).
        # Piece 0 is split 256/256 so pairs 0-1 can start while 2-3 stream.
        w1_t = {}

        def load_w1_cols(lo, hi, tag):
            t = wpool.tile([P, KH, hi - lo], DT, tag=tag, name=tag)
            nc.sync.dma_start(
                out=t[:], in_=w1t_d[:, lo:hi].rearrange("(k p) c -> p k c", p=P))
            return t

        nc.sync.dma_start(out=xt_t[:, KH // 2:, :c1],
                          in_=xt_view[:, KH // 2:, :c1])
        nc.sync.dma_start(out=w1_0a[:, KH // 2:, :],
                          in_=w1_0a_view[:, KH // 2:, :])
        w1_t["0a"] = w1_0a
        w1_t["0b"] = load_w1_cols(2 * P, W1PC, "w1_0b")
        if c1 < c_total:
            nc.sync.dma_start(out=xt_t[:, :, c1:], in_=xt_view[:, :, c1:])
        for piece in (4, 1, 5, 2, 6, 3, 7):
            w1_t[piece] = load_w1_cols(piece * W1PC, (piece + 1) * W1PC,
                                       f"w1_{piece}")

        # w2t: two merged DMAs (8 k-tiles each)
        w2_sb = []
        for half in range(2):
            t = wpool.tile([P, KI // 2, H], dt2, tag=f"w2_{half}")
            rs = slice(half * INTER // 2, (half + 1) * INTER // 2)
            nc.sync.dma_start(
                out=t[:], in_=w2t_d[rs, :].rearrange("(k p) c -> p k c", p=P))
            w2_sb.extend(t[:, k, :] for k in range(KI // 2))

        sc_sb = xpool.tile([P, c_total], f32, tag="sc")
        nc.sync.dma_start(out=sc_sb[:], in_=sc_d[:])

        def w1_slice(k, i):
            # stationary lhsT [P(h), P(inter)] for global inter tile i (0..31)
            piece, sub = divmod(i, W1PC // P)
            if piece == 0:
                if sub < 2:
                    return w1_t["0a"][:, k, P * sub:P * (sub + 1)]
                return w1_t["0b"][:, k, P * (sub - 2):P * (sub - 1)]
            return w1_t[piece][:, k, P * sub:P * (sub + 1)]

        # chunk slices (over the token free dim; PSUM caps a chunk at 512)
        csls = []
        c0 = 0
        for cn in chunk_sizes:
            csls.append((slice(c0, c0 + cn), cn))
            c0 += cn

        # ---- GEMM1 + SwiGLU: yT[i] = silu(gate_i) * up_i, [P, c_total] ----
        # Chunk loop is innermost so each w1 stationary tile is consumed
        # across the full GEMM1 span (halves the required w1 DMA bandwidth).
        # Quad structure (4 gate pairs, then their 4 ups) gives the PE ~8 us
        # of gate work from w1 piece p while up piece p+4 is still in flight.
        yt_sb = [None] * NPAIR
        for q in range(NPAIR // 4):
            quad = range(4 * q, 4 * q + 4)
            sgs = {}
            for i in quad:
                yt_sb[i] = ypool.tile([P, c_total], dt2, tag=f"yt{i}",
                                      name=f"yt{i}")
            for ci, (csl, cn) in enumerate(csls):
                for i in quad:
                    ps_g = pgpool.tile([P, cn], f32, tag="psg")
                    for k in range(KH):
                        nc.tensor.matmul(ps_g[:], lhsT=w1_slice(k, i),
                                         rhs=xt_sb[k][:, csl],
                                         start=(k == 0), stop=(k == KH - 1))
                    sg = apool.tile([P, cn], f32, tag=f"sg{i % 4}_{ci}")
                    nc.scalar.activation(sg[:], ps_g[:],
                                         mybir.ActivationFunctionType.Silu)
                    sgs[(i, ci)] = sg
            for ci, (csl, cn) in enumerate(csls):
                for i in quad:
                    ps_u = pupool.tile([P, cn], f32, tag="psu")
                    for k in range(KH):
                        nc.tensor.matmul(ps_u[:], lhsT=w1_slice(k, i + NPAIR),
                                         rhs=xt_sb[k][:, csl],
                                         start=(k == 0), stop=(k == KH - 1))
                    nc.vector.tensor_mul(yt_sb[i][:, csl], sgs[(i, ci)][:],
                                         ps_u[:])

        # ---- GEMM2 + combine scale ----
        for j in range(NH):
            for csl, cn in csls:
                ps_o = popool.tile([P, cn], f32, tag="pso")
                for k in range(KI):
                    nc.tensor.matmul(ps_o[:], lhsT=w2_sb[k][:, P * j:P * (j + 1)],
                                     rhs=yt_sb[k][:, csl],
                                     start=(k == 0), stop=(k == KI - 1))
                ot = opool.tile([P, cn], f32, tag="ot")
                nc.vector.tensor_mul(ot[:], sc_sb[:, csl], ps_o[:])
                nc.sync.dma_start(out=out_d[P * j:P * (j + 1), csl], in_=ot[:])

    nc.compile()
    return nc


def kernel(hidden_states, w1, w2, router_w):
    x = np.ascontiguousarray(np.asarray(hidden_states, dtype=np.float32)
                             .reshape(T, H))
    w1 = np.asarray(w1, dtype=np.float32)
    w2 = np.asarray(w2, dtype=np.float32)
    router_w = np.asarray(router_w, dtype=np.float32)

    idxs, wts = _route(x, router_w)
    c_total = max(64, -(-max(len(i) for i in idxs) // 4) * 4)

    nc = _PROGRAM_CACHE.get(c_total)
    if nc is None:
        nc = _PROGRAM_CACHE[c_total] = _build_program(c_total)

    xt_f32 = x.T  # [H, T]
    in_maps = []
    for e in range(E):
        n = len(idxs[e])
        xt = np.zeros((H, c_total), dtype=NP_DT)
        xt[:, :n] = xt_f32[:, idxs[e]].astype(NP_DT)
        sc = np.zeros((P, c_total), dtype=np.float32)
        sc[:, :n] = wts[e][None, :]
        in_maps.append({
            "xt": xt,
            "w1t": np.ascontiguousarray(w1[e].T).astype(NP_DT),
            "w2t": np.ascontiguousarray(w2[e].T).astype(
                np.float32 if G2_F32R else NP_DT),
            "scale": sc,
        })

    try:
        res = run_bass_kernel_spmd(nc, in_maps, list(range(N_CORES)))
    except Exception:
        # transient runtime hiccups (e.g. mesh desync on a fresh session)
        # usually clear on retry
        res = run_bass_kernel_spmd(nc, in_maps, list(range(N_CORES)))

    out = np.zeros((T, H), dtype=np.float32)
    for e in range(E):
        n = len(idxs[e])
        if n:
            out[idxs[e]] += res.results[e]["out"][:, :n].T
    return out.reshape(1, T, H)



# revision 9
# speedup vs baseline: 1.0181x; 1.0181x over previous
"""MoE (top-2 of 8 experts, SwiGLU) on 8 Trainium2 NeuronCores.

Expert-parallel with 2-way inter-dim load balancing:

The per-iteration time is PE-bound, and SPMD padding means every core pays
for the HOTTEST expert's token count (538 here vs 512 mean). Instead of one
expert per core, experts are PAIRED hot-with-cold and each pair is split
across two cores by INTER slices:

    core 2p   : slices 0-7  of hot_p  +  slices 0-7  of cold_p
    core 2p+1 : slices 8-15 of hot_p  +  slices 8-15 of cold_p

(an inter "slice" s = gate/up channel block s of w1 + k-tile s of w2; its
GEMM2 output is a full-[H] partial sum, added on the host, which is already
scatter-adding per-expert outputs). Every core holds exactly half of two
experts' weights — same 12.6MB weight DMA as one full expert — and its PE
work is (c_hot + c_cold)/2 tokens-equivalent: 524 vs 538, a 3% cut, plus
fewer matmul instructions (cold groups fit in one <=512 chunk).

GEMM1+GEMM2 both bf16 (rel err ~4e-3 vs the 2e-2 gate), fp32 PSUM.
"""

import sys

sys.path.insert(0, "/opt/trn_rl_repo")

import numpy as np
import ml_dtypes

import concourse.bass as bass  # noqa: F401  (bass must import before tile)
import concourse.tile as tile
from concourse import bacc, mybir
from concourse.bass_utils import run_bass_kernel_spmd

T = 2048
H = 1024
INTER = 2048
E = 8
TOPK = 2
N_CORES = 8
P = 128

DT = mybir.dt.bfloat16
NP_DT = ml_dtypes.bfloat16

_PROGRAM_CACHE = {}

KH = H // P            # 8 k-tiles for GEMM1
NSL = INTER // P // 2  # 8 inter-slices per core per group
NH = H // P            # 8 output h-tiles


def _route(x, router_w):
    gating = (x @ router_w.T).astype(np.float32)
    m = gating.max(axis=1, keepdims=True)
    p = np.exp(gating - m, dtype=np.float32)
    probs = p / p.sum(axis=1, keepdims=True)
    order = np.argsort(-probs, axis=1, kind="stable")
    sel = order[:, :TOPK]
    topw = np.take_along_axis(probs, sel, axis=1)
    idxs, wts = [], []
    for e in range(E):
        m_e = sel == e
        rows = np.nonzero(m_e.any(axis=1))[0]
        idxs.append(rows.astype(np.int64))
        wts.append(topw[m_e].astype(np.float32))
    return idxs, wts


def _chunks(c):
    """Near-equal multiple-of-4 chunks of <=512 (PSUM bank limit)."""
    n = -(-c // 512)
    base = -(-(-(-c // n)) // 4) * 4
    sizes = []
    left = c
    for _ in range(n - 1):
        sizes.append(base)
        left -= base
    sizes.append(left)
    return [s for s in sizes if s > 0]


def _build_program(cA, cB, loop_n=0):
    """SPMD program: two half-expert groups (A: cA tokens, B: cB tokens).

    Each group: 8 gate/up pairs (GEMM1 over full H) -> swiglu -> GEMM2
    over the 8 owned inter k-tiles -> full-[H] partial output."""
    nc = bacc.Bacc("TRN2", target_bir_lowering=False, debug=False,
                   num_devices=N_CORES)
    f32 = mybir.dt.float32

    d = {}
    for g, c in (("A", cA), ("B", cB)):
        d[f"x{g}"] = nc.dram_tensor(f"x{g}", [H, c], DT,
                                    kind="ExternalInput").ap()
        d[f"w1{g}"] = nc.dram_tensor(f"w1{g}", [H, 2 * P * NSL], DT,
                                     kind="ExternalInput").ap()
        d[f"w2{g}"] = nc.dram_tensor(f"w2{g}", [P * NSL, H], DT,
                                     kind="ExternalInput").ap()
        d[f"sc{g}"] = nc.dram_tensor(f"sc{g}", [P, c], f32,
                                     kind="ExternalInput").ap()
        d[f"out{g}"] = nc.dram_tensor(f"out{g}", [H, c], f32,
                                      kind="ExternalOutput").ap()

    from contextlib import ExitStack
    with tile.TileContext(nc) as tc, ExitStack() as ctx:
        wpool = ctx.enter_context(tc.tile_pool(name="weights", bufs=1))
        xpool = ctx.enter_context(tc.tile_pool(name="xt", bufs=1))
        ypool = ctx.enter_context(tc.tile_pool(name="yt", bufs=2))
        apool = ctx.enter_context(tc.tile_pool(name="act", bufs=2))
        opool = ctx.enter_context(tc.tile_pool(name="ot", bufs=2))
        pgpool = ctx.enter_context(tc.tile_pool(name="psg", bufs=3, space="PSUM"))
        pupool = ctx.enter_context(tc.tile_pool(name="psu", bufs=3, space="PSUM"))
        popool = ctx.enter_context(tc.tile_pool(name="pso", bufs=2, space="PSUM"))

        if loop_n:
            ctx.enter_context(tc.For_i(
                0, loop_n, 1, staggered_reset=True,
                hint_engines=(mybir.EngineType.PE, mybir.EngineType.SP,
                              mybir.EngineType.Activation, mybir.EngineType.DVE)))

        # ---- PE warmup (masks each iteration's DMA prologue + clock ramp)
        warm_sb = xpool.tile([P, P], DT, tag="warm")
        nc.vector.memset(warm_sb[:, 0:1], 0.0)
        ps_w = popool.tile([P, P], f32, tag="pso", name="ps_warm")
        for _ in range(44 if not loop_n else 4):
            nc.tensor.matmul(ps_w[:], lhsT=warm_sb[:], rhs=warm_sb[:],
                             start=True, stop=True)

        # ---- DMA loads, in PE consumption order ----
        # Group A first: xA chunk1 + first w1A piece gate the first matmuls.
        tiles = {}

        def load_x(g, c):
            t = xpool.tile([P, KH, c], DT, tag=f"x{g}")
            v = d[f"x{g}"].rearrange("(k p) c -> p k c", p=P)
            cs = _chunks(c)[0]
            nc.sync.dma_start(out=t[:, :, :cs], in_=v[:, :, :cs])
            if cs < c:
                nc.sync.dma_start(out=t[:, :, cs:], in_=v[:, :, cs:])
            tiles[f"x{g}"] = t

        def load_w1_piece(g, lo, hi, tag):
            t = wpool.tile([P, KH, hi - lo], DT, tag=tag, name=tag)
            nc.sync.dma_start(
                out=t[:],
                in_=d[f"w1{g}"][:, lo:hi].rearrange("(k p) c -> p k c", p=P))
            tiles[tag] = t

        W1C = P * NSL  # 1024 gate cols, then 1024 up cols
        load_x("A", cA)
        # A gate piece 0 split small-first so the PE can start early
        load_w1_piece("A", 0, 2 * P, "w1A_g0a")
        load_w1_piece("A", 2 * P, W1C // 2, "w1A_g0b")
        load_x("B", cB)
        load_w1_piece("A", W1C, W1C + W1C // 2, "w1A_u0")
        load_w1_piece("A", W1C // 2, W1C, "w1A_g1")
        load_w1_piece("A", W1C + W1C // 2, 2 * W1C, "w1A_u1")
        for g in ("A", "B"):
            if g == "B":
                load_w1_piece("B", 0, W1C // 2, "w1B_g0")
                load_w1_piece("B", W1C, W1C + W1C // 2, "w1B_u0")
                load_w1_piece("B", W1C // 2, W1C, "w1B_g1")
                load_w1_piece("B", W1C + W1C // 2, 2 * W1C, "w1B_u1")
            t = wpool.tile([P, NSL, H], DT, tag=f"w2{g}")
            nc.sync.dma_start(
                out=t[:], in_=d[f"w2{g}"].rearrange("(k p) c -> p k c", p=P))
            tiles[f"w2{g}"] = t
            ts = xpool.tile([P, cA if g == "A" else cB], f32, tag=f"sc{g}")
            nc.sync.dma_start(out=ts[:], in_=d[f"sc{g}"][:])
            tiles[f"sc{g}"] = ts

        def w1_slice(g, k, i):
            """Stationary lhsT [P(h), P(inter)] for local tile i (0..15):
            i<NSL = gate slice i, else up slice i-NSL."""
            if g == "A":
                if i < NSL:
                    blk, sub = divmod(i, 4)
                    if blk == 0:
                        if sub < 2:
                            return tiles["w1A_g0a"][:, k, P * sub:P * (sub + 1)]
                        return tiles["w1A_g0b"][:, k, P * (sub - 2):P * (sub - 1)]
                    return tiles["w1A_g1"][:, k, P * sub:P * (sub + 1)]
                blk, sub = divmod(i - NSL, 4)
                return tiles[f"w1A_u{blk}"][:, k, P * sub:P * (sub + 1)]
            blk, sub = divmod(i % NSL, 4)
            pre = "g" if i < NSL else "u"
            return tiles[f"w1B_{pre}{blk}"][:, k, P * sub:P * (sub + 1)]

        # ---- per-group pipeline ----
        for g, c in (("A", cA), ("B", cB)):
            x_sb = [tiles[f"x{g}"][:, k, :] for k in range(KH)]
            csls = []
            c0 = 0
            for cn in _chunks(c):
                csls.append((slice(c0, c0 + cn), cn))
                c0 += cn

            yt = ypool.tile([P, NSL, c], DT, tag=f"y{g}")
            for q in range(NSL // 4):
                quad = range(4 * q, 4 * q + 4)
                sgs = {}
                for ci, (csl, cn) in enumerate(csls):
                    for i in quad:
                        ps_g = pgpool.tile([P, cn], f32, tag="psg")
                        for k in range(KH):
                            nc.tensor.matmul(ps_g[:], lhsT=w1_slice(g, k, i),
                                             rhs=x_sb[k][:, csl],
                                             start=(k == 0), stop=(k == KH - 1))
                        sg = apool.tile([P, cn], f32, tag=f"sg{i % 4}_{ci}")
                        nc.scalar.activation(sg[:], ps_g[:],
                                             mybir.ActivationFunctionType.Silu)
                        sgs[(i, ci)] = sg
                for ci, (csl, cn) in enumerate(csls):
                    for i in quad:
                        ps_u = pupool.tile([P, cn], f32, tag="psu")
                        for k in range(KH):
                            nc.tensor.matmul(ps_u[:],
                                             lhsT=w1_slice(g, k, i + NSL),
                                             rhs=x_sb[k][:, csl],
                                             start=(k == 0), stop=(k == KH - 1))
                        nc.vector.tensor_mul(yt[:, i, csl], sgs[(i, ci)][:],
                                             ps_u[:])

            w2t = tiles[f"w2{g}"]
            sc_sb = tiles[f"sc{g}"]
            for j in range(NH):
                for csl, cn in csls:
                    ps_o = popool.tile([P, cn], f32, tag="pso")
                    for k in range(NSL):
                        nc.tensor.matmul(
                            ps_o[:], lhsT=w2t[:, k, P * j:P * (j + 1)],
                            rhs=yt[:, k, csl],
                            start=(k == 0), stop=(k == NSL - 1))
                    ot = opool.tile([P, cn], f32, tag="ot")
                    nc.vector.tensor_mul(ot[:], sc_sb[:, csl], ps_o[:])
                    # NOTE: stores must stay on the SP queue — issuing them
                    # from the ACT HWDGE queue raced the DVE writes here
                    # (garbage output) on this stack.
                    nc.sync.dma_start(out=d[f"out{g}"][P * j:P * (j + 1), csl],
                                      in_=ot[:])

    nc.compile()
    return nc


def _plan(x, w1, w2, router_w):
    """Routing + pairing + per-core quantized input maps."""
    idxs, wts = _route(x, router_w)
    loads = [len(i) for i in idxs]
    order = sorted(range(E), key=lambda e: -loads[e])
    pairs = [(order[p], order[E - 1 - p]) for p in range(E // 2)]
    pad4 = lambda n: max(64, -(-n // 4) * 4)
    cA = pad4(max(loads[a] for a, _ in pairs))
    cB = pad4(max(loads[b] for _, b in pairs))

    xt = x.T  # [H, T]
    halves = {}

    def expert_half(e, half, c):
        n = len(idxs[e])
        xe = np.zeros((H, c), dtype=NP_DT)
        xe[:, :n] = xt[:, idxs[e]].astype(NP_DT)
        w1t = w1[e].T  # [H, 2I] = [gate | up]
        s = slice(half * P * NSL, (half + 1) * P * NSL)
        su = slice(INTER + half * P * NSL, INTER + (half + 1) * P * NSL)
        w1h = np.concatenate([w1t[:, s], w1t[:, su]], axis=1)
        w2h = w2[e].T[half * P * NSL:(half + 1) * P * NSL, :]  # [I/2, H]
        sc = np.zeros((P, c), dtype=np.float32)
        sc[:, :n] = wts[e][None, :]
        return {
            "x": xe,
            "w1": np.ascontiguousarray(w1h).astype(NP_DT),
            "w2": np.ascontiguousarray(w2h).astype(NP_DT),
            "sc": sc,
        }

    in_maps = []
    for a, b in pairs:
        for half in range(2):
            ha = expert_half(a, half, cA)
            hb = expert_half(b, half, cB)
            in_maps.append({
                "xA": ha["x"], "w1A": ha["w1"], "w2A": ha["w2"],
                "scA": ha["sc"],
                "xB": hb["x"], "w1B": hb["w1"], "w2B": hb["w2"],
                "scB": hb["sc"],
            })
    return idxs, pairs, cA, cB, in_maps


def kernel(hidden_states, w1, w2, router_w):
    x = np.ascontiguousarray(np.asarray(hidden_states, dtype=np.float32)
                             .reshape(T, H))
    w1 = np.asarray(w1, dtype=np.float32)
    w2 = np.asarray(w2, dtype=np.float32)
    router_w = np.asarray(router_w, dtype=np.float32)

    idxs, pairs, cA, cB, in_maps = _plan(x, w1, w2, router_w)

    nc = _PROGRAM_CACHE.get((cA, cB))
    if nc is None:
        nc = _PROGRAM_CACHE[(cA, cB)] = _build_program(cA, cB)

    try:
        res = run_bass_kernel_spmd(nc, in_maps, list(range(N_CORES)))
    except Exception:
        res = run_bass_kernel_spmd(nc, in_maps, list(range(N_CORES)))

    out = np.zeros((T, H), dtype=np.float32)
    for p, (a, b) in enumerate(pairs):
        for g, e in (("A", a), ("B", b)):
            n = len(idxs[e])
            if n:
                part = (res.results[2 * p][f"out{g}"][:, :n]
                        + res.results[2 * p + 1][f"out{g}"][:, :n])
                out[idxs[e]] += part.T
    return out.reshape(1, T, H)


# revision 10
# speedup vs baseline: 1.5816x; 1.5534x over previous
"""MoE (top-2 of 8 experts, SwiGLU) on 8 Trainium2 NeuronCores.

Expert-parallel with 2-way inter-dim load balancing:

The per-iteration time is PE-bound, and SPMD padding means every core pays
for the HOTTEST expert's token count (538 here vs 512 mean). Instead of one
expert per core, experts are PAIRED hot-with-cold and each pair is split
across two cores by INTER slices:

    core 2p   : slices 0-7  of hot_p  +  slices 0-7  of cold_p
    core 2p+1 : slices 8-15 of hot_p  +  slices 8-15 of cold_p

(an inter "slice" s = gate/up channel block s of w1 + k-tile s of w2; its
GEMM2 output is a full-[H] partial sum, added on the host, which is already
scatter-adding per-expert outputs). Every core holds exactly half of two
experts' weights — same 12.6MB weight DMA as one full expert — and its PE
work is (c_hot + c_cold)/2 tokens-equivalent: 524 vs 538, a 3% cut, plus
fewer matmul instructions (cold groups fit in one <=512 chunk).

GEMM1+GEMM2 both bf16 (rel err ~4e-3 vs the 2e-2 gate), fp32 PSUM.
"""

import sys

sys.path.insert(0, "/opt/trn_rl_repo")

import numpy as np
import ml_dtypes

import concourse.bass as bass  # noqa: F401  (bass must import before tile)
import concourse.tile as tile
from concourse import bacc, mybir
from concourse.bass_utils import run_bass_kernel_spmd

T = 2048
H = 1024
INTER = 2048
E = 8
TOPK = 2
N_CORES = 8
P = 128

DT = mybir.dt.bfloat16
NP_DT = ml_dtypes.bfloat16

_PROGRAM_CACHE = {}

KH = H // P            # 8 k-tiles for GEMM1
NSL = INTER // P // 2  # 8 inter-slices per core per group
NH = H // P            # 8 output h-tiles


def _route(x, router_w):
    gating = (x @ router_w.T).astype(np.float32)
    m = gating.max(axis=1, keepdims=True)
    p = np.exp(gating - m, dtype=np.float32)
    probs = p / p.sum(axis=1, keepdims=True)
    order = np.argsort(-probs, axis=1, kind="stable")
    sel = order[:, :TOPK]
    topw = np.take_along_axis(probs, sel, axis=1)
    idxs, wts = [], []
    for e in range(E):
        m_e = sel == e
        rows = np.nonzero(m_e.any(axis=1))[0]
        idxs.append(rows.astype(np.int64))
        wts.append(topw[m_e].astype(np.float32))
    return idxs, wts


def _chunks(c):
    """Near-equal multiple-of-4 chunks of <=512 (PSUM bank limit)."""
    n = -(-c // 512)
    base = -(-(-(-c // n)) // 4) * 4
    sizes = []
    left = c
    for _ in range(n - 1):
        sizes.append(base)
        left -= base
    sizes.append(left)
    return [s for s in sizes if s > 0]


def _build_program(cA, cB, loop_n=0):
    """SPMD program: two half-expert groups (A: cA tokens, B: cB tokens).

    Each group: 8 gate/up pairs (GEMM1 over full H) -> swiglu -> GEMM2
    over the 8 owned inter k-tiles -> full-[H] partial output."""
    nc = bacc.Bacc("TRN2", target_bir_lowering=False, debug=False,
                   num_devices=N_CORES)
    f32 = mybir.dt.float32

    d = {}
    for g, c in (("A", cA), ("B", cB)):
        d[f"x{g}"] = nc.dram_tensor(f"x{g}", [H, c], DT,
                                    kind="ExternalInput").ap()
        d[f"w1{g}"] = nc.dram_tensor(f"w1{g}", [H, 2 * P * NSL], DT,
                                     kind="ExternalInput").ap()
        d[f"w2{g}"] = nc.dram_tensor(f"w2{g}", [P * NSL, H], DT,
                                     kind="ExternalInput").ap()
        d[f"sc{g}"] = nc.dram_tensor(f"sc{g}", [P, c], f32,
                                     kind="ExternalInput").ap()
        d[f"out{g}"] = nc.dram_tensor(f"out{g}", [H, c], f32,
                                      kind="ExternalOutput").ap()

    from contextlib import ExitStack
    with tile.TileContext(nc) as tc, ExitStack() as ctx:
        wpool = ctx.enter_context(tc.tile_pool(name="weights", bufs=1))
        xpool = ctx.enter_context(tc.tile_pool(name="xt", bufs=1))
        ypool = ctx.enter_context(tc.tile_pool(name="yt", bufs=2))
        apool = ctx.enter_context(tc.tile_pool(name="act", bufs=2))
        opool = ctx.enter_context(tc.tile_pool(name="ot", bufs=2))
        pgpool = ctx.enter_context(tc.tile_pool(name="psg", bufs=3, space="PSUM"))
        pupool = ctx.enter_context(tc.tile_pool(name="psu", bufs=3, space="PSUM"))
        popool = ctx.enter_context(tc.tile_pool(name="pso", bufs=2, space="PSUM"))

        if loop_n:
            ctx.enter_context(tc.For_i(
                0, loop_n, 1, staggered_reset=True,
                hint_engines=(mybir.EngineType.PE, mybir.EngineType.SP,
                              mybir.EngineType.Activation, mybir.EngineType.DVE)))

        # ---- PE warmup (masks each iteration's DMA prologue + clock ramp)
        warm_sb = xpool.tile([P, P], DT, tag="warm")
        nc.vector.memset(warm_sb[:, 0:1], 0.0)
        ps_w = popool.tile([P, P], f32, tag="pso", name="ps_warm")
        for _ in range(44):
            nc.tensor.matmul(ps_w[:], lhsT=warm_sb[:], rhs=warm_sb[:],
                             start=True, stop=True)

        # ---- DMA loads, in PE consumption order ----
        # Group A first: xA chunk1 + first w1A piece gate the first matmuls.
        tiles = {}

        def load_x(g, c):
            t = xpool.tile([P, KH, c], DT, tag=f"x{g}")
            v = d[f"x{g}"].rearrange("(k p) c -> p k c", p=P)
            cs = _chunks(c)[0]
            nc.sync.dma_start(out=t[:, :, :cs], in_=v[:, :, :cs])
            if cs < c:
                nc.sync.dma_start(out=t[:, :, cs:], in_=v[:, :, cs:])
            tiles[f"x{g}"] = t

        def load_w1_piece(g, lo, hi, tag):
            t = wpool.tile([P, KH, hi - lo], DT, tag=tag, name=tag)
            nc.sync.dma_start(
                out=t[:],
                in_=d[f"w1{g}"][:, lo:hi].rearrange("(k p) c -> p k c", p=P))
            tiles[tag] = t

        W1C = P * NSL  # 1024 gate cols, then 1024 up cols
        load_x("A", cA)
        # A gate piece 0 split small-first so the PE can start early
        load_w1_piece("A", 0, 2 * P, "w1A_g0a")
        load_w1_piece("A", 2 * P, W1C // 2, "w1A_g0b")
        load_x("B", cB)
        load_w1_piece("A", W1C, W1C + W1C // 2, "w1A_u0")
        load_w1_piece("A", W1C // 2, W1C, "w1A_g1")
        load_w1_piece("A", W1C + W1C // 2, 2 * W1C, "w1A_u1")
        for g in ("A", "B"):
            if g == "B":
                load_w1_piece("B", 0, W1C // 2, "w1B_g0")
                load_w1_piece("B", W1C, W1C + W1C // 2, "w1B_u0")
                load_w1_piece("B", W1C // 2, W1C, "w1B_g1")
                load_w1_piece("B", W1C + W1C // 2, 2 * W1C, "w1B_u1")
            t = wpool.tile([P, NSL, H], DT, tag=f"w2{g}")
            nc.sync.dma_start(
                out=t[:], in_=d[f"w2{g}"].rearrange("(k p) c -> p k c", p=P))
            tiles[f"w2{g}"] = t
            ts = xpool.tile([P, cA if g == "A" else cB], f32, tag=f"sc{g}")
            nc.sync.dma_start(out=ts[:], in_=d[f"sc{g}"][:])
            tiles[f"sc{g}"] = ts

        def w1_slice(g, k, i):
            """Stationary lhsT [P(h), P(inter)] for local tile i (0..15):
            i<NSL = gate slice i, else up slice i-NSL."""
            if g == "A":
                if i < NSL:
                    blk, sub = divmod(i, 4)
                    if blk == 0:
                        if sub < 2:
                            return tiles["w1A_g0a"][:, k, P * sub:P * (sub + 1)]
                        return tiles["w1A_g0b"][:, k, P * (sub - 2):P * (sub - 1)]
                    return tiles["w1A_g1"][:, k, P * sub:P * (sub + 1)]
                blk, sub = divmod(i - NSL, 4)
                return tiles[f"w1A_u{blk}"][:, k, P * sub:P * (sub + 1)]
            blk, sub = divmod(i % NSL, 4)
            pre = "g" if i < NSL else "u"
            return tiles[f"w1B_{pre}{blk}"][:, k, P * sub:P * (sub + 1)]

        # ---- per-group pipeline ----
        for g, c in (("A", cA), ("B", cB)):
            x_sb = [tiles[f"x{g}"][:, k, :] for k in range(KH)]
            csls = []
            c0 = 0
            for cn in _chunks(c):
                csls.append((slice(c0, c0 + cn), cn))
                c0 += cn

            yt = ypool.tile([P, NSL, c], DT, tag=f"y{g}")
            for q in range(NSL // 4):
                quad = range(4 * q, 4 * q + 4)
                sgs = {}
                for ci, (csl, cn) in enumerate(csls):
                    for i in quad:
                        ps_g = pgpool.tile([P, cn], f32, tag="psg")
                        for k in range(KH):
                            nc.tensor.matmul(ps_g[:], lhsT=w1_slice(g, k, i),
                                             rhs=x_sb[k][:, csl],
                                             start=(k == 0), stop=(k == KH - 1))
                        sg = apool.tile([P, cn], f32, tag=f"sg{i % 4}_{ci}")
                        nc.scalar.activation(sg[:], ps_g[:],
                                             mybir.ActivationFunctionType.Silu)
                        sgs[(i, ci)] = sg
                for ci, (csl, cn) in enumerate(csls):
                    for i in quad:
                        ps_u = pupool.tile([P, cn], f32, tag="psu")
                        for k in range(KH):
                            nc.tensor.matmul(ps_u[:],
                                             lhsT=w1_slice(g, k, i + NSL),
                                             rhs=x_sb[k][:, csl],
                                             start=(k == 0), stop=(k == KH - 1))
                        nc.vector.tensor_mul(yt[:, i, csl], sgs[(i, ci)][:],
                                             ps_u[:])

            w2t = tiles[f"w2{g}"]
            sc_sb = tiles[f"sc{g}"]
            for j in range(NH):
                for csl, cn in csls:
                    ps_o = popool.tile([P, cn], f32, tag="pso")
                    for k in range(NSL):
                        nc.tensor.matmul(
                            ps_o[:], lhsT=w2t[:, k, P * j:P * (j + 1)],
                            rhs=yt[:, k, csl],
                            start=(k == 0), stop=(k == NSL - 1))
                    ot = opool.tile([P, cn], f32, tag="ot")
                    nc.vector.tensor_mul(ot[:], sc_sb[:, csl], ps_o[:])
                    # NOTE: stores must stay on the SP queue — issuing them
                    # from the ACT HWDGE queue raced the DVE writes here
                    # (garbage output) on this stack.
                    nc.sync.dma_start(out=d[f"out{g}"][P * j:P * (j + 1), csl],
                                      in_=ot[:])

    nc.compile()
    return nc


def _plan(x, w1, w2, router_w):
    """Routing + pairing + per-core quantized input maps."""
    idxs, wts = _route(x, router_w)
    loads = [len(i) for i in idxs]
    order = sorted(range(E), key=lambda e: -loads[e])
    pairs = [(order[p], order[E - 1 - p]) for p in range(E // 2)]
    pad4 = lambda n: max(64, -(-n // 4) * 4)
    cA = pad4(max(loads[a] for a, _ in pairs))
    cB = pad4(max(loads[b] for _, b in pairs))

    xt = x.T  # [H, T]
    halves = {}

    def expert_half(e, half, c):
        n = len(idxs[e])
        xe = np.zeros((H, c), dtype=NP_DT)
        xe[:, :n] = xt[:, idxs[e]].astype(NP_DT)
        w1t = w1[e].T  # [H, 2I] = [gate | up]
        s = slice(half * P * NSL, (half + 1) * P * NSL)
        su = slice(INTER + half * P * NSL, INTER + (half + 1) * P * NSL)
        w1h = np.concatenate([w1t[:, s], w1t[:, su]], axis=1)
        w2h = w2[e].T[half * P * NSL:(half + 1) * P * NSL, :]  # [I/2, H]
        sc = np.zeros((P, c), dtype=np.float32)
        sc[:, :n] = wts[e][None, :]
        return {
            "x": xe,
            "w1": np.ascontiguousarray(w1h).astype(NP_DT),
            "w2": np.ascontiguousarray(w2h).astype(NP_DT),
            "sc": sc,
        }

    in_maps = []
    for a, b in pairs:
        for half in range(2):
            ha = expert_half(a, half, cA)
            hb = expert_half(b, half, cB)
            in_maps.append({
                "xA": ha["x"], "w1A": ha["w1"], "w2A": ha["w2"],
                "scA": ha["sc"],
                "xB": hb["x"], "w1B": hb["w1"], "w2B": hb["w2"],
                "scB": hb["sc"],
            })
    return idxs, pairs, cA, cB, in_maps


def kernel(hidden_states, w1, w2, router_w):
    x = np.ascontiguousarray(np.asarray(hidden_states, dtype=np.float32)
                             .reshape(T, H))
    w1 = np.asarray(w1, dtype=np.float32)
    w2 = np.asarray(w2, dtype=np.float32)
    router_w = np.asarray(router_w, dtype=np.float32)

    idxs, pairs, cA, cB, in_maps = _plan(x, w1, w2, router_w)

    nc = _PROGRAM_CACHE.get((cA, cB))
    if nc is None:
        nc = _PROGRAM_CACHE[(cA, cB)] = _build_program(cA, cB)

    try:
        res = run_bass_kernel_spmd(nc, in_maps, list(range(N_CORES)))
    except Exception:
        res = run_bass_kernel_spmd(nc, in_maps, list(range(N_CORES)))

    out = np.zeros((T, H), dtype=np.float32)
    for p, (a, b) in enumerate(pairs):
        for g, e in (("A", a), ("B", b)):
            n = len(idxs[e])
            if n:
                part = (res.results[2 * p][f"out{g}"][:, :n]
                        + res.results[2 * p + 1][f"out{g}"][:, :n])
                out[idxs[e]] += part.T
    return out.reshape(1, T, H)


# revision 12
# speedup vs baseline: 1.6592x; 1.0490x over previous
"""MoE (top-2 of 8 experts, SwiGLU) on 8 Trainium2 NeuronCores.

Expert-parallel with 2-way inter-dim load balancing:

The per-iteration time is PE-bound, and SPMD padding means every core pays
for the HOTTEST expert's token count (538 here vs 512 mean). Instead of one
expert per core, experts are PAIRED hot-with-cold and each pair is split
across two cores by INTER slices:

    core 2p   : slices 0-7  of hot_p  +  slices 0-7  of cold_p
    core 2p+1 : slices 8-15 of hot_p  +  slices 8-15 of cold_p

(an inter "slice" s = gate/up channel block s of w1 + k-tile s of w2; its
GEMM2 output is a full-[H] partial sum, added on the host, which is already
scatter-adding per-expert outputs). Every core holds exactly half of two
experts' weights — same 12.6MB weight DMA as one full expert — and its PE
work is (c_hot + c_cold)/2 tokens-equivalent: 524 vs 538, a 3% cut, plus
fewer matmul instructions (cold groups fit in one <=512 chunk).

GEMM1+GEMM2 both bf16 (rel err ~4e-3 vs the 2e-2 gate), fp32 PSUM.
"""

import sys

sys.path.insert(0, "/opt/trn_rl_repo")

import numpy as np
import ml_dtypes

import concourse.bass as bass  # noqa: F401  (bass must import before tile)
import concourse.tile as tile
from concourse import bacc, mybir
from concourse.bass_utils import run_bass_kernel_spmd

T = 2048
H = 1024
INTER = 2048
E = 8
TOPK = 2
N_CORES = 8
P = 128

DT = mybir.dt.bfloat16
NP_DT = ml_dtypes.bfloat16

_PROGRAM_CACHE = {}

KH = H // P            # 8 k-tiles for GEMM1
NSL = INTER // P // 2  # 8 inter-slices per core per group
NH = H // P            # 8 output h-tiles


def _route(x, router_w):
    gating = (x @ router_w.T).astype(np.float32)
    m = gating.max(axis=1, keepdims=True)
    p = np.exp(gating - m, dtype=np.float32)
    probs = p / p.sum(axis=1, keepdims=True)
    order = np.argsort(-probs, axis=1, kind="stable")
    sel = order[:, :TOPK]
    topw = np.take_along_axis(probs, sel, axis=1)
    idxs, wts = [], []
    for e in range(E):
        m_e = sel == e
        rows = np.nonzero(m_e.any(axis=1))[0]
        idxs.append(rows.astype(np.int64))
        wts.append(topw[m_e].astype(np.float32))
    return idxs, wts


def _chunks(c):
    """Near-equal multiple-of-4 chunks of <=512 (PSUM bank limit)."""
    n = -(-c // 512)
    base = -(-(-(-c // n)) // 4) * 4
    sizes = []
    left = c
    for _ in range(n - 1):
        sizes.append(base)
        left -= base
    sizes.append(left)
    return [s for s in sizes if s > 0]


def _build_program(cA, cB, loop_n=0):
    """SPMD program: two half-expert groups (A: cA tokens, B: cB tokens).

    Each group: 8 gate/up pairs (GEMM1 over full H) -> swiglu -> GEMM2
    over the 8 owned inter k-tiles -> full-[H] partial output."""
    nc = bacc.Bacc("TRN2", target_bir_lowering=False, debug=False,
                   num_devices=N_CORES)
    f32 = mybir.dt.float32

    d = {}
    for g, c in (("A", cA), ("B", cB)):
        d[f"x{g}"] = nc.dram_tensor(f"x{g}", [H, c], DT,
                                    kind="ExternalInput").ap()
        d[f"w1{g}"] = nc.dram_tensor(f"w1{g}", [H, 2 * P * NSL], DT,
                                     kind="ExternalInput").ap()
        d[f"w2{g}"] = nc.dram_tensor(f"w2{g}", [P * NSL, H], DT,
                                     kind="ExternalInput").ap()
        d[f"sc{g}"] = nc.dram_tensor(f"sc{g}", [P, c], f32,
                                     kind="ExternalInput").ap()
        d[f"out{g}"] = nc.dram_tensor(f"out{g}", [H, c], f32,
                                      kind="ExternalOutput").ap()

    from contextlib import ExitStack
    with tile.TileContext(nc) as tc, ExitStack() as ctx:
        wpool = ctx.enter_context(tc.tile_pool(name="weights", bufs=1))
        xpool = ctx.enter_context(tc.tile_pool(name="xt", bufs=1))
        ypool = ctx.enter_context(tc.tile_pool(name="yt", bufs=2))
        apool = ctx.enter_context(tc.tile_pool(name="act", bufs=2))
        opool = ctx.enter_context(tc.tile_pool(name="ot", bufs=2))
        pgpool = ctx.enter_context(tc.tile_pool(name="psg", bufs=3, space="PSUM"))
        pupool = ctx.enter_context(tc.tile_pool(name="psu", bufs=3, space="PSUM"))
        popool = ctx.enter_context(tc.tile_pool(name="pso", bufs=2, space="PSUM"))

        if loop_n:
            ctx.enter_context(tc.For_i(
                0, loop_n, 1, staggered_reset=True,
                hint_engines=(mybir.EngineType.PE, mybir.EngineType.SP,
                              mybir.EngineType.Activation, mybir.EngineType.DVE)))

        # ---- PE warmup (masks each iteration's DMA prologue + clock ramp)
        warm_sb = xpool.tile([P, P], DT, tag="warm")
        nc.vector.memset(warm_sb[:, 0:1], 0.0)
        ps_w = popool.tile([P, P], f32, tag="pso", name="ps_warm")
        for _ in range(44):
            nc.tensor.matmul(ps_w[:], lhsT=warm_sb[:], rhs=warm_sb[:],
                             start=True, stop=True)

        # ---- DMA loads, in PE consumption order ----
        # Group A first: xA chunk1 + first w1A piece gate the first matmuls.
        tiles = {}

        def load_x(g, c):
            t = xpool.tile([P, KH, c], DT, tag=f"x{g}")
            v = d[f"x{g}"].rearrange("(k p) c -> p k c", p=P)
            cs = _chunks(c)[0]
            nc.sync.dma_start(out=t[:, :, :cs], in_=v[:, :, :cs])
            if cs < c:
                nc.sync.dma_start(out=t[:, :, cs:], in_=v[:, :, cs:])
            tiles[f"x{g}"] = t

        def load_w1_piece(g, lo, hi, tag):
            t = wpool.tile([P, KH, hi - lo], DT, tag=tag, name=tag)
            nc.sync.dma_start(
                out=t[:],
                in_=d[f"w1{g}"][:, lo:hi].rearrange("(k p) c -> p k c", p=P))
            tiles[tag] = t

        W1C = P * NSL  # 1024 gate cols, then 1024 up cols
        load_x("A", cA)
        # A gate piece 0 split small-first so the PE can start early
        load_w1_piece("A", 0, 2 * P, "w1A_g0a")
        load_w1_piece("A", 2 * P, W1C // 2, "w1A_g0b")
        load_x("B", cB)
        load_w1_piece("A", W1C, W1C + W1C // 2, "w1A_u0")
        load_w1_piece("A", W1C // 2, W1C, "w1A_g1")
        load_w1_piece("A", W1C + W1C // 2, 2 * W1C, "w1A_u1")
        for g in ("A", "B"):
            if g == "B":
                load_w1_piece("B", 0, W1C // 2, "w1B_g0")
                load_w1_piece("B", W1C, W1C + W1C // 2, "w1B_u0")
                load_w1_piece("B", W1C // 2, W1C, "w1B_g1")
                load_w1_piece("B", W1C + W1C // 2, 2 * W1C, "w1B_u1")
            t = wpool.tile([P, NSL, H], DT, tag=f"w2{g}")
            nc.sync.dma_start(
                out=t[:], in_=d[f"w2{g}"].rearrange("(k p) c -> p k c", p=P))
            tiles[f"w2{g}"] = t
            ts = xpool.tile([P, cA if g == "A" else cB], f32, tag=f"sc{g}")
            nc.sync.dma_start(out=ts[:], in_=d[f"sc{g}"][:])
            tiles[f"sc{g}"] = ts

        def w1_slice(g, k, i):
            """Stationary lhsT [P(h), P(inter)] for local tile i (0..15):
            i<NSL = gate slice i, else up slice i-NSL."""
            if g == "A":
                if i < NSL:
                    blk, sub = divmod(i, 4)
                    if blk == 0:
                        if sub < 2:
                            return tiles["w1A_g0a"][:, k, P * sub:P * (sub + 1)]
                        return tiles["w1A_g0b"][:, k, P * (sub - 2):P * (sub - 1)]
                    return tiles["w1A_g1"][:, k, P * sub:P * (sub + 1)]
                blk, sub = divmod(i - NSL, 4)
                return tiles[f"w1A_u{blk}"][:, k, P * sub:P * (sub + 1)]
            blk, sub = divmod(i % NSL, 4)
            pre = "g" if i < NSL else "u"
            return tiles[f"w1B_{pre}{blk}"][:, k, P * sub:P * (sub + 1)]

        # ---- per-group pipeline ----
        for g, c in (("A", cA), ("B", cB)):
            x_sb = [tiles[f"x{g}"][:, k, :] for k in range(KH)]
            csls = []
            c0 = 0
            for cn in _chunks(c):
                csls.append((slice(c0, c0 + cn), cn))
                c0 += cn

            yt = ypool.tile([P, NSL, c], DT, tag=f"y{g}")
            for q in range(NSL // 4):
                quad = range(4 * q, 4 * q + 4)
                sgs = {}
                for ci, (csl, cn) in enumerate(csls):
                    for i in quad:
                        ps_g = pgpool.tile([P, cn], f32, tag="psg")
                        for k in range(KH):
                            nc.tensor.matmul(ps_g[:], lhsT=w1_slice(g, k, i),
                                             rhs=x_sb[k][:, csl],
                                             start=(k == 0), stop=(k == KH - 1))
                        sg = apool.tile([P, cn], f32, tag=f"sg{i % 4}_{ci}")
                        nc.scalar.activation(sg[:], ps_g[:],
                                             mybir.ActivationFunctionType.Silu)
                        sgs[(i, ci)] = sg
                for ci, (csl, cn) in enumerate(csls):
                    for i in quad:
                        ps_u = pupool.tile([P, cn], f32, tag="psu")
                        for k in range(KH):
                            nc.tensor.matmul(ps_u[:],
                                             lhsT=w1_slice(g, k, i + NSL),
                                             rhs=x_sb[k][:, csl],
                                             start=(k == 0), stop=(k == KH - 1))
                        nc.vector.tensor_mul(yt[:, i, csl], sgs[(i, ci)][:],
                                             ps_u[:])

            w2t = tiles[f"w2{g}"]
            sc_sb = tiles[f"sc{g}"]
            for j in range(NH):
                for csl, cn in csls:
                    ps_o = popool.tile([P, cn], f32, tag="pso")
                    for k in range(NSL):
                        nc.tensor.matmul(
                            ps_o[:], lhsT=w2t[:, k, P * j:P * (j + 1)],
                            rhs=yt[:, k, csl],
                            start=(k == 0), stop=(k == NSL - 1))
                    ot = opool.tile([P, cn], f32, tag="ot")
                    nc.vector.tensor_mul(ot[:], sc_sb[:, csl], ps_o[:])
                    # NOTE: stores must stay on the SP queue — issuing them
                    # from the ACT HWDGE queue raced the DVE writes here
                    # (garbage output) on this stack.
                    nc.sync.dma_start(out=d[f"out{g}"][P * j:P * (j + 1), csl],
                                      in_=ot[:])

    nc.compile()
    return nc


def _plan(x, w1, w2, router_w):
    """Routing + pairing + per-core quantized input maps."""
    idxs, wts = _route(x, router_w)
    loads = [len(i) for i in idxs]
    order = sorted(range(E), key=lambda e: -loads[e])
    pairs = [(order[p], order[E - 1 - p]) for p in range(E // 2)]
    pad4 = lambda n: max(64, -(-n // 4) * 4)
    cA = pad4(max(loads[a] for a, _ in pairs))
    cB = pad4(max(loads[b] for _, b in pairs))

    xt = x.T  # [H, T]
    halves = {}

    def expert_half(e, half, c):
        n = len(idxs[e])
        xe = np.zeros((H, c), dtype=NP_DT)
        xe[:, :n] = xt[:, idxs[e]].astype(NP_DT)
        w1t = w1[e].T  # [H, 2I] = [gate | up]
        s = slice(half * P * NSL, (half + 1) * P * NSL)
        su = slice(INTER + half * P * NSL, INTER + (half + 1) * P * NSL)
        w1h = np.concatenate([w1t[:, s], w1t[:, su]], axis=1)
        w2h = w2[e].T[half * P * NSL:(half + 1) * P * NSL, :]  # [I/2, H]
        sc = np.zeros((P, c), dtype=np.float32)
        sc[:, :n] = wts[e][None, :]
        return {
            "x": xe,
            "w1": np.ascontiguousarray(w1h).astype(NP_DT),
            "w2": np.ascontiguousarray(w2h).astype(NP_DT),
            "sc": sc,
        }

    in_maps = []
    for a, b in pairs:
        for half in range(2):
            ha = expert_half(a, half, cA)
            hb = expert_half(b, half, cB)
            in_maps.append({
                "xA": ha["x"], "w1A": ha["w1"], "w2A": ha["w2"],
                "scA": ha["sc"],
                "xB": hb["x"], "w1B": hb["w1"], "w2B": hb["w2"],
                "scB": hb["sc"],
            })
    return idxs, pairs, cA, cB, in_maps


def kernel(hidden_states, w1, w2, router_w):
    x = np.ascontiguousarray(np.asarray(hidden_states, dtype=np.float32)
                             .reshape(T, H))
    w1 = np.asarray(w1, dtype=np.float32)
    w2 = np.asarray(w2, dtype=np.float32)
    router_w = np.asarray(router_w, dtype=np.float32)

    idxs, pairs, cA, cB, in_maps = _plan(x, w1, w2, router_w)

    nc = _PROGRAM_CACHE.get((cA, cB))
    if nc is None:
        nc = _PROGRAM_CACHE[(cA, cB)] = _build_program(cA, cB)

    try:
        res = run_bass_kernel_spmd(nc, in_maps, list(range(N_CORES)))
    except Exception:
        res = run_bass_kernel_spmd(nc, in_maps, list(range(N_CORES)))

    out = np.zeros((T, H), dtype=np.float32)
    for p, (a, b) in enumerate(pairs):
        for g, e in (("A", a), ("B", b)):
            n = len(idxs[e])
            if n:
                part = (res.results[2 * p][f"out{g}"][:, :n]
                        + res.results[2 * p + 1][f"out{g}"][:, :n])
                out[idxs[e]] += part.T
    return out.reshape(1, T, H)


# revision 13
# speedup vs baseline: 1.6985x; 1.0237x over previous
"""MoE (top-2 of 8 experts, SwiGLU) on 8 Trainium2 NeuronCores.

Expert-parallel with 2-way inter-dim load balancing:

The per-iteration time is PE-bound, and SPMD padding means every core pays
for the HOTTEST expert's token count (538 here vs 512 mean). Instead of one
expert per core, experts are PAIRED hot-with-cold and each pair is split
across two cores by INTER slices:

    core 2p   : slices 0-7  of hot_p  +  slices 0-7  of cold_p
    core 2p+1 : slices 8-15 of hot_p  +  slices 8-15 of cold_p

(an inter "slice" s = gate/up channel block s of w1 + k-tile s of w2; its
GEMM2 output is a full-[H] partial sum, added on the host, which is already
scatter-adding per-expert outputs). Every core holds exactly half of two
experts' weights — same 12.6MB weight DMA as one full expert — and its PE
work is (c_hot + c_cold)/2 tokens-equivalent: 524 vs 538, a 3% cut, plus
fewer matmul instructions (cold groups fit in one <=512 chunk).

GEMM1+GEMM2 both bf16 (rel err ~4e-3 vs the 2e-2 gate), fp32 PSUM.
"""

import sys

sys.path.insert(0, "/opt/trn_rl_repo")

import numpy as np
import ml_dtypes

import concourse.bass as bass  # noqa: F401  (bass must import before tile)
import concourse.tile as tile
from concourse import bacc, mybir
from concourse.bass_utils import run_bass_kernel_spmd

T = 2048
H = 1024
INTER = 2048
E = 8
TOPK = 2
N_CORES = 8
P = 128

DT = mybir.dt.bfloat16
NP_DT = ml_dtypes.bfloat16

_PROGRAM_CACHE = {}

KH = H // P            # 8 k-tiles for GEMM1
NSL = INTER // P // 2  # 8 inter-slices per core per group
NH = H // P            # 8 output h-tiles


def _route(x, router_w):
    gating = (x @ router_w.T).astype(np.float32)
    m = gating.max(axis=1, keepdims=True)
    p = np.exp(gating - m, dtype=np.float32)
    probs = p / p.sum(axis=1, keepdims=True)
    order = np.argsort(-probs, axis=1, kind="stable")
    sel = order[:, :TOPK]
    topw = np.take_along_axis(probs, sel, axis=1)
    idxs, wts = [], []
    for e in range(E):
        m_e = sel == e
        rows = np.nonzero(m_e.any(axis=1))[0]
        idxs.append(rows.astype(np.int64))
        wts.append(topw[m_e].astype(np.float32))
    return idxs, wts


def _chunks(c):
    """Near-equal multiple-of-4 chunks of <=512 (PSUM bank limit)."""
    n = -(-c // 512)
    base = -(-(-(-c // n)) // 4) * 4
    sizes = []
    left = c
    for _ in range(n - 1):
        sizes.append(base)
        left -= base
    sizes.append(left)
    return [s for s in sizes if s > 0]


def _build_program(cA, cB, loop_n=0):
    """SPMD program: two half-expert groups (A: cA tokens, B: cB tokens).

    Each group: 8 gate/up pairs (GEMM1 over full H) -> swiglu -> GEMM2
    over the 8 owned inter k-tiles -> full-[H] partial output."""
    nc = bacc.Bacc("TRN2", target_bir_lowering=False, debug=False,
                   num_devices=N_CORES)
    f32 = mybir.dt.float32

    d = {}
    for g, c in (("A", cA), ("B", cB)):
        d[f"x{g}"] = nc.dram_tensor(f"x{g}", [H, c], DT,
                                    kind="ExternalInput").ap()
        d[f"w1{g}"] = nc.dram_tensor(f"w1{g}", [H, 2 * P * NSL], DT,
                                     kind="ExternalInput").ap()
        d[f"w2{g}"] = nc.dram_tensor(f"w2{g}", [P * NSL, H], DT,
                                     kind="ExternalInput").ap()
        d[f"sc{g}"] = nc.dram_tensor(f"sc{g}", [P, c], f32,
                                     kind="ExternalInput").ap()
        # bf16 partials: halves the store bytes on the SP queue, whose
        # FIFO order would otherwise delay the next iteration's prefetch
        # (adds ~2e-4 rel err on top of 4e-3 — negligible vs the 2e-2 gate)
        d[f"out{g}"] = nc.dram_tensor(f"out{g}", [H, c], DT,
                                      kind="ExternalOutput").ap()

    from contextlib import ExitStack
    with tile.TileContext(nc) as tc, ExitStack() as ctx:
        wpool = ctx.enter_context(tc.tile_pool(name="weights", bufs=1))
        xpool = ctx.enter_context(tc.tile_pool(name="xt", bufs=1))
        ypool = ctx.enter_context(tc.tile_pool(name="yt", bufs=2))
        apool = ctx.enter_context(tc.tile_pool(name="act", bufs=2))
        opool = ctx.enter_context(tc.tile_pool(name="ot", bufs=2))
        pgpool = ctx.enter_context(tc.tile_pool(name="psg", bufs=3, space="PSUM"))
        pupool = ctx.enter_context(tc.tile_pool(name="psu", bufs=3, space="PSUM"))
        popool = ctx.enter_context(tc.tile_pool(name="pso", bufs=2, space="PSUM"))

        if loop_n:
            ctx.enter_context(tc.For_i(
                0, loop_n, 1, staggered_reset=True,
                hint_engines=(mybir.EngineType.PE, mybir.EngineType.SP,
                              mybir.EngineType.Activation, mybir.EngineType.DVE)))

        # ---- PE warmup (masks each iteration's DMA prologue + clock ramp)
        warm_sb = xpool.tile([P, P], DT, tag="warm")
        nc.vector.memset(warm_sb[:, 0:1], 0.0)
        ps_w = popool.tile([P, P], f32, tag="pso", name="ps_warm")
        for _ in range(44):
            nc.tensor.matmul(ps_w[:], lhsT=warm_sb[:], rhs=warm_sb[:],
                             start=True, stop=True)

        # ---- DMA loads, in PE consumption order ----
        # Group A first: xA chunk1 + first w1A piece gate the first matmuls.
        tiles = {}

        def load_x(g, c):
            t = xpool.tile([P, KH, c], DT, tag=f"x{g}")
            v = d[f"x{g}"].rearrange("(k p) c -> p k c", p=P)
            cs = _chunks(c)[0]
            nc.sync.dma_start(out=t[:, :, :cs], in_=v[:, :, :cs])
            if cs < c:
                nc.sync.dma_start(out=t[:, :, cs:], in_=v[:, :, cs:])
            tiles[f"x{g}"] = t

        def load_w1_piece(g, lo, hi, tag):
            t = wpool.tile([P, KH, hi - lo], DT, tag=tag, name=tag)
            nc.sync.dma_start(
                out=t[:],
                in_=d[f"w1{g}"][:, lo:hi].rearrange("(k p) c -> p k c", p=P))
            tiles[tag] = t

        W1C = P * NSL  # 1024 gate cols, then 1024 up cols
        load_x("A", cA)
        # A gate piece 0 split small-first so the PE can start early
        load_w1_piece("A", 0, 2 * P, "w1A_g0a")
        load_w1_piece("A", 2 * P, W1C // 2, "w1A_g0b")
        load_x("B", cB)
        load_w1_piece("A", W1C, W1C + W1C // 2, "w1A_u0")
        load_w1_piece("A", W1C // 2, W1C, "w1A_g1")
        load_w1_piece("A", W1C + W1C // 2, 2 * W1C, "w1A_u1")
        for g in ("A", "B"):
            if g == "B":
                load_w1_piece("B", 0, W1C // 2, "w1B_g0")
                load_w1_piece("B", W1C, W1C + W1C // 2, "w1B_u0")
                load_w1_piece("B", W1C // 2, W1C, "w1B_g1")
                load_w1_piece("B", W1C + W1C // 2, 2 * W1C, "w1B_u1")
            t = wpool.tile([P, NSL, H], DT, tag=f"w2{g}")
            nc.sync.dma_start(
                out=t[:], in_=d[f"w2{g}"].rearrange("(k p) c -> p k c", p=P))
            tiles[f"w2{g}"] = t
            ts = xpool.tile([P, cA if g == "A" else cB], f32, tag=f"sc{g}")
            nc.sync.dma_start(out=ts[:], in_=d[f"sc{g}"][:])
            tiles[f"sc{g}"] = ts

        def w1_slice(g, k, i):
            """Stationary lhsT [P(h), P(inter)] for local tile i (0..15):
            i<NSL = gate slice i, else up slice i-NSL."""
            if g == "A":
                if i < NSL:
                    blk, sub = divmod(i, 4)
                    if blk == 0:
                        if sub < 2:
                            return tiles["w1A_g0a"][:, k, P * sub:P * (sub + 1)]
                        return tiles["w1A_g0b"][:, k, P * (sub - 2):P * (sub - 1)]
                    return tiles["w1A_g1"][:, k, P * sub:P * (sub + 1)]
                blk, sub = divmod(i - NSL, 4)
                return tiles[f"w1A_u{blk}"][:, k, P * sub:P * (sub + 1)]
            blk, sub = divmod(i % NSL, 4)
            pre = "g" if i < NSL else "u"
            return tiles[f"w1B_{pre}{blk}"][:, k, P * sub:P * (sub + 1)]

        # ---- per-group pipeline ----
        for g, c in (("A", cA), ("B", cB)):
            x_sb = [tiles[f"x{g}"][:, k, :] for k in range(KH)]
            csls = []
            c0 = 0
            for cn in _chunks(c):
                csls.append((slice(c0, c0 + cn), cn))
                c0 += cn

            yt = ypool.tile([P, NSL, c], DT, tag=f"y{g}")
            for q in range(NSL // 4):
                quad = range(4 * q, 4 * q + 4)
                sgs = {}
                for ci, (csl, cn) in enumerate(csls):
                    for i in quad:
                        ps_g = pgpool.tile([P, cn], f32, tag="psg")
                        for k in range(KH):
                            nc.tensor.matmul(ps_g[:], lhsT=w1_slice(g, k, i),
                                             rhs=x_sb[k][:, csl],
                                             start=(k == 0), stop=(k == KH - 1))
                        sg = apool.tile([P, cn], f32, tag=f"sg{i % 4}_{ci}")
                        nc.scalar.activation(sg[:], ps_g[:],
                                             mybir.ActivationFunctionType.Silu)
                        sgs[(i, ci)] = sg
                for ci, (csl, cn) in enumerate(csls):
                    for i in quad:
                        ps_u = pupool.tile([P, cn], f32, tag="psu")
                        for k in range(KH):
                            nc.tensor.matmul(ps_u[:],
                                             lhsT=w1_slice(g, k, i + NSL),
                                             rhs=x_sb[k][:, csl],
                                             start=(k == 0), stop=(k == KH - 1))
                        nc.vector.tensor_mul(yt[:, i, csl], sgs[(i, ci)][:],
                                             ps_u[:])

            w2t = tiles[f"w2{g}"]
            sc_sb = tiles[f"sc{g}"]
            for j in range(NH):
                for csl, cn in csls:
                    ps_o = popool.tile([P, cn], f32, tag="pso")
                    for k in range(NSL):
                        nc.tensor.matmul(
                            ps_o[:], lhsT=w2t[:, k, P * j:P * (j + 1)],
                            rhs=yt[:, k, csl],
                            start=(k == 0), stop=(k == NSL - 1))
                    ot = opool.tile([P, cn], DT, tag="ot")
                    nc.vector.tensor_mul(ot[:], sc_sb[:, csl], ps_o[:])
                    # NOTE: stores must stay on the SP queue — issuing them
                    # from the ACT HWDGE queue raced the DVE writes here
                    # (garbage output) on this stack.
                    nc.sync.dma_start(out=d[f"out{g}"][P * j:P * (j + 1), csl],
                                      in_=ot[:])

    nc.compile()
    return nc


def _plan(x, w1, w2, router_w):
    """Routing + pairing + per-core quantized input maps."""
    idxs, wts = _route(x, router_w)
    loads = [len(i) for i in idxs]
    order = sorted(range(E), key=lambda e: -loads[e])
    pairs = [(order[p], order[E - 1 - p]) for p in range(E // 2)]
    pad4 = lambda n: max(64, -(-n // 4) * 4)
    cA = pad4(max(loads[a] for a, _ in pairs))
    cB = pad4(max(loads[b] for _, b in pairs))

    xt = x.T  # [H, T]
    halves = {}

    def expert_half(e, half, c):
        n = len(idxs[e])
        xe = np.zeros((H, c), dtype=NP_DT)
        xe[:, :n] = xt[:, idxs[e]].astype(NP_DT)
        w1t = w1[e].T  # [H, 2I] = [gate | up]
        s = slice(half * P * NSL, (half + 1) * P * NSL)
        su = slice(INTER + half * P * NSL, INTER + (half + 1) * P * NSL)
        w1h = np.concatenate([w1t[:, s], w1t[:, su]], axis=1)
        w2h = w2[e].T[half * P * NSL:(half + 1) * P * NSL, :]  # [I/2, H]
        sc = np.zeros((P, c), dtype=np.float32)
        sc[:, :n] = wts[e][None, :]
        return {
            "x": xe,
            "w1": np.ascontiguousarray(w1h).astype(NP_DT),
            "w2": np.ascontiguousarray(w2h).astype(NP_DT),
            "sc": sc,
        }

    in_maps = []
    for a, b in pairs:
        for half in range(2):
            ha = expert_half(a, half, cA)
            hb = expert_half(b, half, cB)
            in_maps.append({
                "xA": ha["x"], "w1A": ha["w1"], "w2A": ha["w2"],
                "scA": ha["sc"],
                "xB": hb["x"], "w1B": hb["w1"], "w2B": hb["w2"],
                "scB": hb["sc"],
            })
    return idxs, pairs, cA, cB, in_maps


def kernel(hidden_states, w1, w2, router_w):
    x = np.ascontiguousarray(np.asarray(hidden_states, dtype=np.float32)
                             .reshape(T, H))
    w1 = np.asarray(w1, dtype=np.float32)
    w2 = np.asarray(w2, dtype=np.float32)
    router_w = np.asarray(router_w, dtype=np.float32)

    idxs, pairs, cA, cB, in_maps = _plan(x, w1, w2, router_w)

    nc = _PROGRAM_CACHE.get((cA, cB))
    if nc is None:
        nc = _PROGRAM_CACHE[(cA, cB)] = _build_program(cA, cB)

    try:
        res = run_bass_kernel_spmd(nc, in_maps, list(range(N_CORES)))
    except Exception:
        res = run_bass_kernel_spmd(nc, in_maps, list(range(N_CORES)))

    out = np.zeros((T, H), dtype=np.float32)
    for p, (a, b) in enumerate(pairs):
        for g, e in (("A", a), ("B", b)):
            n = len(idxs[e])
            if n:
                part = (res.results[2 * p][f"out{g}"][:, :n]
                        .astype(np.float32)
                        + res.results[2 * p + 1][f"out{g}"][:, :n]
                        .astype(np.float32))
                out[idxs[e]] += part.T
    return out.reshape(1, T, H)
